# revision 47
# baseline (speedup 1.0000x reference)
"""Distributed MultiHeadAttention (+residual, +LayerNorm) Trainium2 kernel.

Problem: B=2, S=2048, D_MODEL=1024, N_HEAD=16, D_K=D_V=64, eps=1e-6.
  qh = q@Wq, kh = k@Wk, vh = v@Wv  (per head)
  attn = softmax(qh·kh^T / 8)
  out = (attn@vh) @ Wfc + bfc + q  -> LayerNorm(gamma, beta)

Sharding: 8 cores; core c owns 512 q-rows of batch c//4 (sequence shard).
Each core projects K/V for its own 512 rows; AllGathers over each 4-core
batch group materialize the full-batch K^T/V; attention, fc and
LayerNorm are then fully local.

v2 schedule (from trace analysis of the baseline):
  - the serial CC core boots ~55us into the kernel and then executes
    meshes back-to-back; compute must never wait on a piece it needs
    early.  The 5 AllGathers are trigger-chained (nosync deps) in
    consumption order: wu warmup, K-half0, V-half0, K-half1, V-half1.
  - the exp stream on ACT is the critical resource (160 exps of
    [128,1024] ~= 184us).  It is fed as early as possible: projections
    run K -> Q -> V(k-subs 0,1), then the local attention pass over own
    k-cols 0:255 starts while V(k-subs 2,3) projects, then local pass
    over own k-cols 256:511.  The remote passes chew the gathered
    pieces the moment they land.
  - DMA queues: sync = kT/vT + gathered kh + vh(s01) + out; scalar =
    Wk/Wv + constants; gpsimd = qT/Wq + collective staging/triggers +
    vh(s23) + Wfc/qn.
Own-chunk contributions are counted once: the gathered passes mask the
core's own chunk to exp()=0 via a per-core -30000 bias input.
"""

import sys

sys.path.insert(0, "/opt/trn_rl_repo")

import ml_dtypes
import numpy as np

import contextlib

import concourse.bass as bass
import concourse.tile as tile
from concourse import bacc, mybir
from concourse.bass import _add_dep_helper
from concourse.bass_utils import run_bass_kernel_spmd

N_CORES = 8
B = 2
S = 2048
D = 1024  # d_model
H = 16  # heads
DK = 64  # head dim
SS = S // 4  # 512 q-rows per core
HS = SS // 2  # 256 k-columns per K collective piece
LN_EPS = 1e-6
F32 = mybir.dt.float32
F16 = mybir.dt.float16
BF16 = mybir.dt.bfloat16
F8 = mybir.dt.float8e4

VW = H * 65  # vh row width with per-head ones column baked in


def build_kernel():
    nc = bacc.Bacc()

    qT = nc.dram_tensor("qT", [D, SS], BF16, kind="ExternalInput")
    kT = nc.dram_tensor("kT", [D, SS], BF16, kind="ExternalInput")
    vT = nc.dram_tensor("vT", [D, SS], BF16, kind="ExternalInput")
    qn = nc.dram_tensor("qn", [SS, D], F32, kind="ExternalInput")  # residual + bfc
    Wq = nc.dram_tensor("Wq", [D, D], BF16, kind="ExternalInput")
    Wk = nc.dram_tensor("Wk", [D, D], BF16, kind="ExternalInput")
    Wv = nc.dram_tensor("Wv", [D, D], BF16, kind="ExternalInput")
    Wfc = nc.dram_tensor("Wfc", [D, D], BF16, kind="ExternalInput")
    gb = nc.dram_tensor("gb", [128, D], F32, kind="ExternalInput")  # gamma bcast
    bb = nc.dram_tensor("bb", [128, D], F32, kind="ExternalInput")  # beta bcast
    mb = nc.dram_tensor("mb", [128, 4], F32, kind="ExternalInput")  # own-chunk mask
    out = nc.dram_tensor("out", [SS, D], F32, kind="ExternalOutput")

    # collective staging, split into consumption-ordered pieces.
    # K pieces are k-column halves (all head-pairs usable per piece).
    ck_in = [nc.dram_tensor(f"ck{p}_in", [8 * 128, HS], BF16) for p in range(2)]
    ck_out = [nc.dram_tensor(f"ck{p}_out", [32 * 128, HS], BF16) for p in range(2)]
    cv_in = [nc.dram_tensor(f"cv{p}_in", [2 * 128, VW], F8) for p in range(2)]
    cv_out = [nc.dram_tensor(f"cv{p}_out", [8 * 128, VW], F8) for p in range(2)]
    rk = nc.dram_tensor("rk", [1, 4], mybir.dt.int32, kind="ExternalInput")
    eye = nc.dram_tensor("eye", [128, 128], F32, kind="ExternalInput")
    wu_in = nc.dram_tensor("wu_in", [128], BF16)
    wu_out = nc.dram_tensor("wu_out", [512], BF16)

    def ck_in_v(p, i):  # k-half p, khT d-chunk i (0..7) -> [128, 256] view
        return ck_in[p][128 * i : 128 * (i + 1), :]

    def ck_out_v(p, rc8, i):  # k-half p, chunk-row reg, d-chunk i -> [128, 256]
        # rc8 holds c*1024 (the gathered chunk's row offset)
        return ck_out[p][128 * i :][bass.ds(rc8, 128), :]

    def cv_in_v(s):  # vh k-sub s (0..3) -> [128, 1040] staging view
        p, l = divmod(s, 2)
        return cv_in[p][128 * l : 128 * (l + 1), :]

    def cv_out_v(rc2, s):  # chunk-row reg (c*256), k-sub s -> [128, 1040]
        p, l = divmod(s, 2)
        return cv_out[p][128 * l :][bass.ds(rc2, 128), :]

    env = dict(locals())
    with tile.TileContext(nc) as tc:
        _build_body(nc, tc, env)
    nc.compile()
    return nc


def _build_body(nc, tc, env):
    qT = env["qT"]; kT = env["kT"]; vT = env["vT"]; qn = env["qn"]
    Wq = env["Wq"]; Wk = env["Wk"]; Wv = env["Wv"]; Wfc = env["Wfc"]
    gb = env["gb"]; bb = env["bb"]; mb = env["mb"]; out = env["out"]
    ck_in = env["ck_in"]; ck_out = env["ck_out"]
    cv_in = env["cv_in"]; cv_out = env["cv_out"]
    ck_in_v = env["ck_in_v"]; ck_out_v = env["ck_out_v"]
    cv_in_v = env["cv_in_v"]; cv_out_v = env["cv_out_v"]
    RG = [[0, 1, 2, 3], [4, 5, 6, 7]]

    cc_chain = []

    def ag(tin, tout):
        cc = nc.gpsimd.collective_compute(
            "AllGather", mybir.AluOpType.bypass, replica_groups=RG,
            ins=[tin[:]], outs=[tout[:]],
        )
        if cc_chain:
            _add_dep_helper(
                cc.ins, cc_chain[-1].ins, sync=False, reason="cc issue order"
            )
        cc_chain.append(cc)

    with contextlib.ExitStack() as stack:
        ep = stack.enter_context
        # persistent SBUF (fresh addresses, no WAR with phase-A pools)
        qhT_pool = ep(tc.tile_pool(name="qhT", bufs=1))
        stagek_pool = ep(tc.tile_pool(name="stagek", bufs=1))
        stagev_pool = ep(tc.tile_pool(name="stagev", bufs=1))
        outT_pool = ep(tc.tile_pool(name="outT", bufs=1))
        vh_pool = ep(tc.tile_pool(name="vh", bufs=1))
        khr_pool = ep(tc.tile_pool(name="khr", bufs=8))
        pt_pool = ep(tc.tile_pool(name="pt", bufs=26))
        pol_pool = ep(tc.tile_pool(name="pol", bufs=1))
        cst_pool = ep(tc.tile_pool(name="cst", bufs=1))
        dn_pool = ep(tc.tile_pool(name="dn", bufs=1))
        # collective warmup: tiny AllGather triggered before anything else
        # (the CC core takes ~55us to boot; this starts that clock)
        ag(env["wu_in"], env["wu_out"])

        # ---- constants (scalar queue, behind nothing critical yet) ------
        mbt = cst_pool.tile([128, 4], F32, tag="mb")
        gbt = cst_pool.tile([128, D], F32, tag="gb")
        bbt = cst_pool.tile([128, D], F32, tag="bb")
        eyet = cst_pool.tile([128, 128], F32, tag="eye")
        epst = cst_pool.tile([128, 1], F32, tag="eps")
        nc.vector.memset(epst[:], LN_EPS)

        # zero-padded per-sub q moving tiles (zeros written now, the live
        # halves copied in after the Q projection) -> every attention
        # matmul runs with a uniform full-128-row stationary tile config
        qhT_tiles = []
        for dchunk in range(8):
            qsubs = [
                qhT_pool.tile(
                    [128, SS], BF16, tag=f"qh{dchunk}_{sub}",
                    name=f"qh{dchunk}_{sub}",
                )
                for sub in range(2)
            ]
            nc.vector.memset(qsubs[0][64:128, :], 0.0)
            nc.vector.memset(qsubs[1][0:64, :], 0.0)
            qhT_tiles.append(qsubs)

        # ================ Phase A: QKV projections ======================
        stack_a = contextlib.ExitStack()
        if True:
            epa = stack_a.enter_context
            xin_pool = epa(tc.tile_pool(name="xin", bufs=1))
            wk_pool = epa(tc.tile_pool(name="wk", bufs=1))
            wv_pool = epa(tc.tile_pool(name="wv", bufs=1))
            wq_pool = epa(tc.tile_pool(name="wq", bufs=1))
            ppv_pool = epa(tc.tile_pool(name="ppv", bufs=1, space="PSUM"))
            pp_stack = contextlib.ExitStack()
            pp_pool = pp_stack.enter_context(
                tc.tile_pool(name="pp", bufs=1, space="PSUM")
            )
            # kT/vT on sync, qT on gpsimd (kT reuses slots with nothing)
            def x_load(eng, srct, slot, tag):
                t = xin_pool.tile([128, SS], BF16, tag=f"x{slot}", name=f"x{tag}")
                eng.dma_start(t[:], srct)
                return t

            kT_t = [x_load(nc.sync, kT[128 * i : 128 * (i + 1), :], i, f"k{i}")
                    for i in range(8)]
            qT_t = [x_load(nc.gpsimd, qT[128 * i : 128 * (i + 1), :], 8 + i,
                           f"q{i}") for i in range(8)]
            vT_t = [x_load(nc.sync, vT[128 * i : 128 * (i + 1), :], i, f"v{i}")
                    for i in range(8)]

            # Wk/Wv on scalar (ACT idle until the exp stream), Wq on gpsimd
            def wk_load(db, i):
                t = wk_pool.tile([128, 512], BF16, tag=f"wk{i}", name=f"wk{i}")
                nc.scalar.dma_start(
                    t[:], Wk[128 * i : 128 * (i + 1), 512 * db : 512 * (db + 1)]
                )
                return t

            wk_t = [wk_load(0, i) for i in range(8)]
            wk1_t = [wk_load(1, i) for i in range(8)]
            wv_t = []
            for i in range(8):
                t = wv_pool.tile([128, D], BF16, tag=f"wv{i}")
                nc.scalar.dma_start(t[:], Wv[128 * i : 128 * (i + 1), :])
                wv_t.append(t)

            # LN constants behind the weights on scalar (needed late)
            nc.scalar.dma_start(mbt[:], mb[:])
            nc.scalar.dma_start(eyet[:], env["eye"][:])
            nc.scalar.dma_start(gbt[:], gb[:])
            nc.scalar.dma_start(bbt[:], bb[:])

            def wq_load(db, i):
                t = wq_pool.tile([128, 512], BF16, tag=f"wq{i}", name=f"wq{i}")
                nc.gpsimd.dma_start(
                    t[:], Wq[128 * i : 128 * (i + 1), 512 * db : 512 * (db + 1)]
                )
                return t

            wq_t = [wq_load(0, i) for i in range(8)]

            # ---- K projection: khT_local[d, k]; CC pieces = k-col halves
            stagek_t = []
            for db in range(2):
                pss = [
                    pp_pool.tile([128, SS], F32, tag=f"pp{d}", name=f"pp{d}")
                    for d in range(4)
                ]
                for i in range(8):
                    wt = wk_t[i] if db == 0 else wk1_t[i]
                    for dsub in range(4):
                        nc.tensor.matmul(
                            pss[dsub][:], wt[:, 128 * dsub : 128 * (dsub + 1)],
                            kT_t[i][:], start=(i == 0), stop=(i == 7),
                        )
                for dsub in range(4):
                    dchunk = 4 * db + dsub
                    st = stagek_pool.tile([128, SS], BF16, tag=f"sk{dchunk}")
                    stagek_t.append(st)
                    nc.vector.tensor_copy(st[:], pss[dsub][:])
                    nc.gpsimd.dma_start(ck_in_v(0, dchunk), st[:, 0:HS])
                    nc.gpsimd.dma_start(ck_in_v(1, dchunk), st[:, HS:SS])
            # (K AllGathers are triggered after cv0's: chain order is
            # wu, V-half0, K-half0, K-half1, V-half1 so the first V piece
            # lands before the first remote PV needs it)

            # ---- Q projection ------------------------------------------
            for db in range(2):
                if db == 1:
                    wq_t = [wq_load(1, i) for i in range(8)]
                pss = [
                    pp_pool.tile([128, SS], F32, tag=f"pp{d}", name=f"pp{d}")
                    for d in range(4)
                ]
                for i in range(8):
                    for dsub in range(4):
                        nc.tensor.matmul(
                            pss[dsub][:],
                            wq_t[i][:, 128 * dsub : 128 * (dsub + 1)],
                            qT_t[i][:], start=(i == 0), stop=(i == 7),
                        )
                for dsub in range(4):
                    dchunk = 4 * db + dsub
                    qsubs = qhT_tiles[dchunk]
                    nc.vector.tensor_copy(qsubs[0][0:64, :], pss[dsub][0:64, :])
                    nc.vector.tensor_copy(
                        qsubs[1][64:128, :], pss[dsub][64:128, :]
                    )

            # pp_pool (K/Q projection PSUM) closes here so the local-pass
            # score PSUM fits alongside the V projection's ppv
            pp_stack.close()

            # ---- V projection, s-group sg: k-subs {2sg, 2sg+1} ---------
            stagev_t = [
                stagev_pool.tile([128, VW], F8, tag=f"sv{s}", name=f"sv{s}")
                for s in range(4)
            ]

            def v_proj(sg):
                for half in range(2):  # hd half: heads 8*half..
                    pvs = [
                        ppv_pool.tile(
                            [128, 512], F32, tag=f"pv{si}",
                            name=f"pv{2 * sg + si}_{half}",
                        )
                        for si in range(2)
                    ]
                    for i in range(8):
                        wm = wv_t[i][:, 512 * half : 512 * (half + 1)]
                        for si in range(2):
                            s = 2 * sg + si
                            nc.tensor.matmul(
                                pvs[si][:], vT_t[i][:, 128 * s : 128 * (s + 1)],
                                wm, start=(i == 0), stop=(i == 7),
                            )
                    for si in range(2):
                        s = 2 * sg + si
                        std = stagev_t[s][:].rearrange("p (h e) -> p h e", e=65)
                        nc.vector.tensor_copy(
                            std[:, 8 * half : 8 * (half + 1), 0:64],
                            pvs[si][:].rearrange("p (h e) -> p h e", e=64),
                        )
                for si in range(2):
                    s = 2 * sg + si
                    std = stagev_t[s][:].rearrange("p (h e) -> p h e", e=65)
                    nc.vector.memset(std[:, :, 64], 1.0)
                    nc.gpsimd.dma_start(cv_in_v(s), stagev_t[s][:])

            v_proj(0)
            ag(ck_in[0], ck_out[0])  # mesh 2: K k-cols 0:256
            ag(cv_in[0], cv_out[0])  # mesh 3: V k-subs 0,1
            ag(ck_in[1], ck_out[1])  # mesh 4: K k-cols 256:512

            # ================ Phase B: attention ========================
            # (opened while phase-A pools are live: the local pass halves
            # interleave with the V sg1 projection)
            pol_t = {}
            pol_gen = {}
            with contextlib.ExitStack() as stack_b:
                epb = stack_b.enter_context
                ps_pool = epb(tc.tile_pool(name="ps", bufs=2, space="PSUM"))
                po_pool = epb(tc.tile_pool(name="po", bufs=1, space="PSUM"))

                def score_pv(hp, kstat2, vstat2, bias, pos, first, last):
                    # both subs of one chunk-pair: 4 score MMs, 2 exps,
                    # 4 PV MMs, grouped by PE tile shape (all 128x128)
                    pss, pts = [], []
                    for sub in range(2):
                        qmov = qhT_tiles[hp][sub]
                        ps = ps_pool.tile([128, 2 * SS], F32, tag="ps", name="ps")
                        pss.append(ps)
                        for u in range(2):
                            nc.tensor.matmul(
                                ps[:, SS * u : SS * (u + 1)], kstat2[u], qmov,
                                start=True, stop=True,
                            )
                    for sub in range(2):
                        pt = pt_pool.tile([128, 2 * SS], F8, tag="pt", name="pt")
                        pts.append(pt)
                        if bias is None:
                            nc.scalar.activation(
                                pts[sub][:], pss[sub][:],
                                mybir.ActivationFunctionType.Exp, scale=0.125,
                            )
                        else:
                            nc.scalar.activation(
                                pts[sub][:], pss[sub][:],
                                mybir.ActivationFunctionType.Exp, scale=0.125,
                                bias=bias,
                            )
                    for sub in range(2):
                        h = 2 * hp + sub
                        for u in range(2):
                            nc.tensor.matmul(
                                pos[sub][:], vstat2[u][:, 65 * h : 65 * h + 65],
                                pts[sub][:, SS * u : SS * (u + 1)],
                                start=(first and u == 0), stop=(last and u == 1),
                            )

                def park(hp, sub, pos):
                    # partial -> SBUF f16; generations alternate between
                    # two tile tags, accumulating in place
                    prev = pol_t.get((hp, sub))
                    gen = pol_gen.get((hp, sub), 0)
                    ab = "AB"[gen % 2]
                    pl = pol_pool.tile(
                        [65, SS], F16, tag=f"pol{ab}{hp}_{sub}",
                        name=f"pol{ab}{hp}_{sub}",
                    )
                    if prev is None:
                        nc.vector.tensor_copy(pl[:], pos[:])
                    else:
                        nc.vector.tensor_add(pl[:], pos[:], prev[:])
                    pol_t[(hp, sub)] = pl
                    pol_gen[(hp, sub)] = gen + 1

                outT_tiles = []
                for i in range(8):
                    oT = outT_pool.tile([128, SS], BF16, tag=f"oT{i}")
                    outT_tiles.append(oT)

                def finish(hp, sub, pos):
                    # combine with parked partials, normalize, write outT
                    pl = pol_t[(hp, sub)]
                    tot = dn_pool.tile(
                        [65, SS], F32, tag=f"tot{sub}", name=f"tot{sub}"
                    )
                    nc.vector.tensor_add(tot[:], pos[:], pl[:])
                    rec = dn_pool.tile(
                        [1, SS], F32, tag=f"rec{sub}", name=f"rec{sub}"
                    )
                    nc.vector.tensor_copy(rec[:], tot[64:65, :])
                    rc2 = dn_pool.tile(
                        [1, SS], F32, tag=f"rc2{sub}", name=f"rc2{sub}"
                    )
                    nc.vector.reciprocal_approx_fast(rc2[:], rec[:])
                    rb = dn_pool.tile([64, SS], F32, tag=f"rb{sub}",
                                      name=f"rb{sub}")
                    nc.gpsimd.partition_broadcast(rb[:], rc2[:])
                    nc.vector.tensor_mul(
                        outT_tiles[hp][64 * sub : 64 * sub + 64, :],
                        tot[0:64, :], rb[:],
                    )

                def local_pass(sg):
                    # own-chunk attention over k-cols [256*sg, 256*sg+256)
                    for hp in range(H // 2):
                        pos = [
                            po_pool.tile(
                                [65, SS], F32, tag=f"po{s}", name=f"po{s}"
                            )
                            for s in range(2)
                        ]
                        kst = [
                            stagek_t[hp][:, 128 * (2 * sg + u) :
                                          128 * (2 * sg + u) + 128]
                            for u in range(2)
                        ]
                        vst = [stagev_t[2 * sg + u][:] for u in range(2)]
                        score_pv(hp, kst, vst, None, pos, True, True)
                        for sub in range(2):
                            park(hp, sub, pos[sub])

                # local attention on k-subs 0,1 overlaps the V sg1 MMs
                local_pass(0)
                v_proj(1)
                ag(cv_in[1], cv_out[1])  # mesh 5: V k-subs 2,3
                local_pass(1)

            stack_a.close()  # phase-A SBUF freed for wfc/resq
            with contextlib.ExitStack() as stack_b:
                epb = stack_b.enter_context
                wfc_pool = epb(tc.tile_pool(name="wfc", bufs=1))
                resq_pool = epb(tc.tile_pool(name="resq", bufs=1))
                psb_stack = contextlib.ExitStack()
                ps_pool = psb_stack.enter_context(
                    tc.tile_pool(name="ps", bufs=3, space="PSUM")
                )
                po_pool = psb_stack.enter_context(
                    tc.tile_pool(name="po", bufs=1, space="PSUM")
                )
                # the gathered passes only touch the 3 REMOTE chunks:
                # chunk index c = (rank + j) % 4 for j=1..3, addressed via
                # sync-engine registers loaded from the per-core rank input
                rrank = nc.sync.alloc_register("rrank")
                nc.sync.reg_load(rrank, env["rk"][0:1, 0:1])
                rk8 = []  # c*1024: ck_out row offset of remote chunk j
                rv2 = []  # c*256: cv_out row offset of remote chunk j
                for j in range(1, 4):
                    r8 = nc.sync.alloc_register(f"rk8_{j}")
                    nc.sync.reg_alu(r8, rrank, j, mybir.AluOpType.add)
                    nc.sync.reg_alu(r8, r8, 3, mybir.AluOpType.bitwise_and)
                    r2 = nc.sync.alloc_register(f"rv2_{j}")
                    nc.sync.reg_alu(r2, r8, 256, mybir.AluOpType.mult)
                    nc.sync.reg_alu(r8, r8, 1024, mybir.AluOpType.mult)
                    rk8.append(nc.sync.snap(r8))
                    rv2.append(nc.sync.snap(r2))

                vh_t = {}

                def vh_load(eng, j, s):
                    t = vh_pool.tile(
                        [128, VW], F8, tag=f"vh{j}_{s}", name=f"vh{j}_{s}"
                    )
                    eng.dma_start(t[:], cv_out_v(rv2[j - 1], s))
                    vh_t[(j, s)] = t

                def khr_load(p, hp):
                    ts = {}
                    for j in range(1, 4):
                        t = khr_pool.tile(
                            [128, HS], BF16, tag=f"khc{j}", name=f"khc{j}"
                        )
                        nc.sync.dma_start(t[:], ck_out_v(p, rk8[j - 1], hp))
                        ts[j] = t
                    return ts

                # sync-queue load order tracks the mesh chain: kh for the
                # first head-pairs (mesh k0), then pass-0 vh (mesh v0),
                # then the rest -- so no load head-of-line blocks another
                # that could already run, and PV stalls stay tiny.
                khs = [khr_load(0, hp) for hp in range(4)]
                for j in range(1, 4):  # pass-0 vh tiles (k-subs 0,1)
                    for s in range(2):
                        vh_load(nc.sync, j, s)
                khs += [khr_load(0, hp) for hp in range(4, 8)]
                for j in range(1, 4):  # pass-1 vh tiles (k-subs 2,3)
                    for s in range(2, 4):
                        vh_load(nc.sync, j, s)

                # wfc / qn prefetch on gpsimd behind the vh loads
                wfc_t = []
                for i in range(8):
                    t = wfc_pool.tile([128, D], BF16, tag=f"wfc{i}")
                    nc.gpsimd.dma_start(t[:], Wfc[128 * i : 128 * (i + 1), :])
                    wfc_t.append(t)
                qn_t = []
                for qs in range(4):
                    t = resq_pool.tile([128, D], F32, tag=f"qn{qs}")
                    nc.gpsimd.dma_start(t[:], qn[128 * qs : 128 * (qs + 1), :])
                    qn_t.append(t)

                for p in range(2):
                    for hp in range(H // 2):
                        kh_t = khs[hp] if p == 0 else khr_load(p, hp)

                        pos = [
                            po_pool.tile([65, SS], F32, tag=f"po{s}",
                                         name=f"po{s}")
                            for s in range(2)
                        ]
                        for j in range(1, 4):
                            kst = [kh_t[j][:, 128 * u : 128 * (u + 1)]
                                   for u in range(2)]
                            vst = [vh_t[(j, 2 * p + u)][:] for u in range(2)]
                            score_pv(
                                hp, kst, vst, None,
                                pos, first=(j == 1), last=(j == 3),
                            )
                        for sub in range(2):
                            if p == 0:
                                park(hp, sub, pos[sub])
                            else:
                                finish(hp, sub, pos[sub])

                # ============ Phase C: fc + residual + LayerNorm ========
                psb_stack.close()  # attention PSUM freed for the fc psums
                with contextlib.ExitStack() as stack_c:
                    epc = stack_c.enter_context
                    pfc_pool = epc(tc.tile_pool(name="pfc", bufs=1, space="PSUM"))
                    lns_pool = epc(tc.tile_pool(name="lns", bufs=1))
                    lnsc_pool = epc(tc.tile_pool(name="lnsc", bufs=1))
                    # stage-major emission: all four q-subtiles advance
                    # together so the per-subtile serial chain (fc -> add ->
                    # mean -> var -> rstd -> scale) pipelines across ACT/DVE
                    pf_l, x_l, nmu_l, rstd_l = [], [], [], []

                    def fc_qs(qs):
                        # fc + residual: the residual rows ride into the
                        # same PSUM accumulation as an identity matmul --
                        # the PE is idle here and it deletes a serial
                        # [128,1024] DVE add per q-subtile
                        pf = pfc_pool.tile([128, D], F32, tag=f"pf{qs}")
                        for i in range(8):
                            stat = outT_tiles[i][:, 128 * qs : 128 * (qs + 1)]
                            nc.tensor.matmul(
                                pf[:, 0:512], stat, wfc_t[i][:, 0:512],
                                start=(i == 0), stop=False,
                            )
                            nc.tensor.matmul(
                                pf[:, 512:1024], stat, wfc_t[i][:, 512:1024],
                                start=(i == 0), stop=False,
                            )
                        for h2 in range(2):
                            nc.tensor.matmul(
                                pf[:, 512 * h2 : 512 * (h2 + 1)], eyet[:],
                                qn_t[qs][:, 512 * h2 : 512 * (h2 + 1)],
                                start=False, stop=True,
                            )
                        pf_l.append(pf)
                        x_l.append(pf)

                    # q-subtile 0's chain leads so the LN ACT stream starts
                    # while the other three fc groups still run on the PE
                    for qs in range(4):
                        fc_qs(qs)
                    for qs in range(4):
                        msum = lnsc_pool.tile([128, 1], F32, tag=f"msum{qs}")
                        nc.vector.reduce_sum(
                            out=msum[:], in_=x_l[qs][:], axis=mybir.AxisListType.X
                        )
                        nmu = lnsc_pool.tile([128, 1], F32, tag=f"nmu{qs}")
                        nc.scalar.activation(
                            nmu[:], msum[:], mybir.ActivationFunctionType.Copy,
                            scale=-1.0 / D,
                        )
                        nmu_l.append(nmu)
                    for qs in range(4):
                        sq = lns_pool.tile([128, D], F32, tag="t", name="sq")
                        vsum = lnsc_pool.tile([128, 1], F32, tag=f"vsum{qs}")
                        nc.scalar.activation(
                            sq[:], x_l[qs][:],
                            mybir.ActivationFunctionType.Square,
                            bias=nmu_l[qs][:], accum_out=vsum[:],
                        )
                        std = lnsc_pool.tile([128, 1], F32, tag=f"std{qs}")
                        nc.scalar.activation(
                            std[:], vsum[:],
                            mybir.ActivationFunctionType.Sqrt,
                            scale=1.0 / D, bias=epst[:],
                        )
                        rstd = lnsc_pool.tile([128, 1], F32, tag=f"rstd{qs}")
                        nc.vector.reciprocal(rstd[:], std[:])
                        rstd_l.append(rstd)
                    for qs in range(4):
                        xn = lns_pool.tile([128, D], F32, tag="t", name="xn")
                        nc.vector.tensor_scalar(
                            out=xn[:], in0=x_l[qs][:], scalar1=nmu_l[qs][:],
                            scalar2=rstd_l[qs][:],
                            op0=mybir.AluOpType.add, op1=mybir.AluOpType.mult,
                        )
                        xg = lns_pool.tile([128, D], F32, tag="g", name="xg")
                        nc.vector.tensor_mul(xg[:], xn[:], gbt[:])
                        xb = lns_pool.tile([128, D], F32, tag="b", name="xb")
                        nc.vector.tensor_add(xb[:], xg[:], bbt[:])
                        nc.sync.dma_start(
                            out[128 * qs : 128 * (qs + 1), :], xb[:]
                        )

_NC_CACHE = None


def kernel(q, k, v, Wq, Wk, Wv, Wfc, bfc, gamma, beta):
    global _NC_CACHE
    if _NC_CACHE is None:
        _NC_CACHE = build_kernel()
    nc = _NC_CACHE

    bf16 = ml_dtypes.bfloat16
    q = np.asarray(q, dtype=np.float32)
    k = np.asarray(k, dtype=np.float32)
    v = np.asarray(v, dtype=np.float32)
    Wq = np.ascontiguousarray(np.asarray(Wq, dtype=np.float32).astype(bf16))
    Wk = np.ascontiguousarray(np.asarray(Wk, dtype=np.float32).astype(bf16))
    Wv = np.ascontiguousarray(np.asarray(Wv, dtype=np.float32).astype(bf16))
    Wfc = np.ascontiguousarray(np.asarray(Wfc, dtype=np.float32).astype(bf16))
    bfc = np.asarray(bfc, dtype=np.float32)
    gamma = np.asarray(gamma, dtype=np.float32)
    beta = np.asarray(beta, dtype=np.float32)

    gb = np.ascontiguousarray(np.broadcast_to(gamma, (128, D)))
    eye = np.eye(128, dtype=np.float32)
    bb = np.ascontiguousarray(np.broadcast_to(beta, (128, D)))

    in_maps = []
    for c in range(N_CORES):
        b, r0 = c // 4, (c % 4) * SS
        qs = q[b, r0 : r0 + SS]
        ks = k[b, r0 : r0 + SS]
        vs = v[b, r0 : r0 + SS]
        mbm = np.zeros((128, 4), np.float32)
        mbm[:, c % 4] = -30000.0
        in_maps.append(
            {
                "rk": np.array([[c % 4, 0, 0, 0]], dtype=np.int32),
                "eye": eye,
                "qT": np.ascontiguousarray(qs.T.astype(bf16)),
                "kT": np.ascontiguousarray(ks.T.astype(bf16)),
                "vT": np.ascontiguousarray(vs.T.astype(bf16)),
                "qn": np.ascontiguousarray(qs + bfc),
                "Wq": Wq, "Wk": Wk, "Wv": Wv, "Wfc": Wfc,
                "gb": gb, "bb": bb, "mb": mbm,
            }
        )

    global _last_in_maps
    _last_in_maps = in_maps
    res = run_bass_kernel_spmd(nc, in_maps, list(range(N_CORES)))
    out = np.empty((B, S, D), dtype=np.float32)
    for c in range(N_CORES):
        b, r0 = c // 4, (c % 4) * SS
        out[b, r0 : r0 + SS] = res.results[c]["out"]
    return out


# revision 48
# speedup vs baseline: 1.0063x; 1.0063x over previous
"""Distributed MultiHeadAttention (+residual, +LayerNorm) Trainium2 kernel.

Problem: B=2, S=2048, D_MODEL=1024, N_HEAD=16, D_K=D_V=64, eps=1e-6.
  qh = q@Wq, kh = k@Wk, vh = v@Wv  (per head)
  attn = softmax(qh·kh^T / 8)
  out = (attn@vh) @ Wfc + bfc + q  -> LayerNorm(gamma, beta)

Sharding: 8 cores; core c owns 512 q-rows of batch c//4 (sequence shard).
Each core projects K/V for its own 512 rows; AllGathers over each 4-core
batch group materialize the full-batch K^T/V; attention, fc and
LayerNorm are then fully local.

v2 schedule (from trace analysis of the baseline):
  - the serial CC core boots ~55us into the kernel and then executes
    meshes back-to-back; compute must never wait on a piece it needs
    early.  The 5 AllGathers are trigger-chained (nosync deps) in
    consumption order: wu warmup, K-half0, V-half0, K-half1, V-half1.
  - the exp stream on ACT is the critical resource (160 exps of
    [128,1024] ~= 184us).  It is fed as early as possible: projections
    run K -> Q -> V(k-subs 0,1), then the local attention pass over own
    k-cols 0:255 starts while V(k-subs 2,3) projects, then local pass
    over own k-cols 256:511.  The remote passes chew the gathered
    pieces the moment they land.
  - DMA queues: sync = kT/vT + gathered kh + vh(s01) + out; scalar =
    Wk/Wv + constants; gpsimd = qT/Wq + collective staging/triggers +
    vh(s23) + Wfc/qn.
Own-chunk contributions are counted once: the gathered passes mask the
core's own chunk to exp()=0 via a per-core -30000 bias input.
"""

import sys

sys.path.insert(0, "/opt/trn_rl_repo")

import ml_dtypes
import numpy as np

import contextlib

import concourse.bass as bass
import concourse.tile as tile
from concourse import bacc, mybir
from concourse.bass import _add_dep_helper
from concourse.bass_utils import run_bass_kernel_spmd

N_CORES = 8
B = 2
S = 2048
D = 1024  # d_model
H = 16  # heads
DK = 64  # head dim
SS = S // 4  # 512 q-rows per core
HS = SS // 2  # 256 k-columns per K collective piece
LN_EPS = 1e-6
F32 = mybir.dt.float32
F16 = mybir.dt.float16
BF16 = mybir.dt.bfloat16
F8 = mybir.dt.float8e4

VW = H * 65  # vh row width with per-head ones column baked in


def build_kernel():
    nc = bacc.Bacc()

    qT = nc.dram_tensor("qT", [D, SS], BF16, kind="ExternalInput")
    kT = nc.dram_tensor("kT", [D, SS], BF16, kind="ExternalInput")
    vT = nc.dram_tensor("vT", [D, SS], BF16, kind="ExternalInput")
    qn = nc.dram_tensor("qn", [SS, D], BF16, kind="ExternalInput")  # residual + bfc
    Wq = nc.dram_tensor("Wq", [D, D], BF16, kind="ExternalInput")
    Wk = nc.dram_tensor("Wk", [D, D], BF16, kind="ExternalInput")
    Wv = nc.dram_tensor("Wv", [D, D], BF16, kind="ExternalInput")
    Wfc = nc.dram_tensor("Wfc", [D, D], BF16, kind="ExternalInput")
    gb = nc.dram_tensor("gb", [128, D], F32, kind="ExternalInput")  # gamma bcast
    bb = nc.dram_tensor("bb", [128, D], F32, kind="ExternalInput")  # beta bcast
    mb = nc.dram_tensor("mb", [128, 4], F32, kind="ExternalInput")  # own-chunk mask
    out = nc.dram_tensor("out", [SS, D], F32, kind="ExternalOutput")

    # collective staging, split into consumption-ordered pieces.
    # K pieces are k-column halves (all head-pairs usable per piece).
    ck_in = [nc.dram_tensor(f"ck{p}_in", [8 * 128, HS], BF16) for p in range(2)]
    ck_out = [nc.dram_tensor(f"ck{p}_out", [32 * 128, HS], BF16) for p in range(2)]
    cv_in = [nc.dram_tensor(f"cv{p}_in", [2 * 128, VW], F8) for p in range(2)]
    cv_out = [nc.dram_tensor(f"cv{p}_out", [8 * 128, VW], F8) for p in range(2)]
    rk = nc.dram_tensor("rk", [1, 4], mybir.dt.int32, kind="ExternalInput")
    eye = nc.dram_tensor("eye", [128, 128], BF16, kind="ExternalInput")
    wu_in = nc.dram_tensor("wu_in", [128], BF16)
    wu_out = nc.dram_tensor("wu_out", [512], BF16)

    def ck_in_v(p, i):  # k-half p, khT d-chunk i (0..7) -> [128, 256] view
        return ck_in[p][128 * i : 128 * (i + 1), :]

    def ck_out_v(p, rc8, i):  # k-half p, chunk-row reg, d-chunk i -> [128, 256]
        # rc8 holds c*1024 (the gathered chunk's row offset)
        return ck_out[p][128 * i :][bass.ds(rc8, 128), :]

    def cv_in_v(s):  # vh k-sub s (0..3) -> [128, 1040] staging view
        p, l = divmod(s, 2)
        return cv_in[p][128 * l : 128 * (l + 1), :]

    def cv_out_v(rc2, s):  # chunk-row reg (c*256), k-sub s -> [128, 1040]
        p, l = divmod(s, 2)
        return cv_out[p][128 * l :][bass.ds(rc2, 128), :]

    env = dict(locals())
    with tile.TileContext(nc) as tc:
        _build_body(nc, tc, env)
    nc.compile()
    return nc


def _build_body(nc, tc, env):
    qT = env["qT"]; kT = env["kT"]; vT = env["vT"]; qn = env["qn"]
    Wq = env["Wq"]; Wk = env["Wk"]; Wv = env["Wv"]; Wfc = env["Wfc"]
    gb = env["gb"]; bb = env["bb"]; mb = env["mb"]; out = env["out"]
    ck_in = env["ck_in"]; ck_out = env["ck_out"]
    cv_in = env["cv_in"]; cv_out = env["cv_out"]
    ck_in_v = env["ck_in_v"]; ck_out_v = env["ck_out_v"]
    cv_in_v = env["cv_in_v"]; cv_out_v = env["cv_out_v"]
    RG = [[0, 1, 2, 3], [4, 5, 6, 7]]

    cc_chain = []

    def ag(tin, tout):
        cc = nc.gpsimd.collective_compute(
            "AllGather", mybir.AluOpType.bypass, replica_groups=RG,
            ins=[tin[:]], outs=[tout[:]],
        )
        if cc_chain:
            _add_dep_helper(
                cc.ins, cc_chain[-1].ins, sync=False, reason="cc issue order"
            )
        cc_chain.append(cc)

    with contextlib.ExitStack() as stack:
        ep = stack.enter_context
        # persistent SBUF (fresh addresses, no WAR with phase-A pools)
        qhT_pool = ep(tc.tile_pool(name="qhT", bufs=1))
        stagek_pool = ep(tc.tile_pool(name="stagek", bufs=1))
        stagev_pool = ep(tc.tile_pool(name="stagev", bufs=1))
        outT_pool = ep(tc.tile_pool(name="outT", bufs=1))
        vh_pool = ep(tc.tile_pool(name="vh", bufs=1))
        khr_pool = ep(tc.tile_pool(name="khr", bufs=8))
        pt_pool = ep(tc.tile_pool(name="pt", bufs=26))
        pol_pool = ep(tc.tile_pool(name="pol", bufs=1))
        cst_pool = ep(tc.tile_pool(name="cst", bufs=1))
        dn_pool = ep(tc.tile_pool(name="dn", bufs=1))
        # collective warmup: tiny AllGather triggered before anything else
        # (the CC core takes ~55us to boot; this starts that clock)
        ag(env["wu_in"], env["wu_out"])

        # ---- constants (scalar queue, behind nothing critical yet) ------
        mbt = cst_pool.tile([128, 4], F32, tag="mb")
        gbt = cst_pool.tile([128, D], F32, tag="gb")
        bbt = cst_pool.tile([128, D], F32, tag="bb")
        eyet = cst_pool.tile([128, 128], BF16, tag="eye")
        epst = cst_pool.tile([128, 1], F32, tag="eps")
        nc.vector.memset(epst[:], LN_EPS)

        # zero-padded per-sub q moving tiles (zeros written now, the live
        # halves copied in after the Q projection) -> every attention
        # matmul runs with a uniform full-128-row stationary tile config
        qhT_tiles = []
        for dchunk in range(8):
            qsubs = [
                qhT_pool.tile(
                    [128, SS], BF16, tag=f"qh{dchunk}_{sub}",
                    name=f"qh{dchunk}_{sub}",
                )
                for sub in range(2)
            ]
            nc.vector.memset(qsubs[0][64:128, :], 0.0)
            nc.vector.memset(qsubs[1][0:64, :], 0.0)
            qhT_tiles.append(qsubs)

        # ================ Phase A: QKV projections ======================
        stack_a = contextlib.ExitStack()
        if True:
            epa = stack_a.enter_context
            xin_pool = epa(tc.tile_pool(name="xin", bufs=1))
            wk_pool = epa(tc.tile_pool(name="wk", bufs=1))
            wv_pool = epa(tc.tile_pool(name="wv", bufs=1))
            wq_pool = epa(tc.tile_pool(name="wq", bufs=1))
            ppv_pool = epa(tc.tile_pool(name="ppv", bufs=1, space="PSUM"))
            pp_stack = contextlib.ExitStack()
            pp_pool = pp_stack.enter_context(
                tc.tile_pool(name="pp", bufs=1, space="PSUM")
            )
            # kT/vT on sync, qT on gpsimd (kT reuses slots with nothing)
            def x_load(eng, srct, slot, tag):
                t = xin_pool.tile([128, SS], BF16, tag=f"x{slot}", name=f"x{tag}")
                eng.dma_start(t[:], srct)
                return t

            kT_t = [x_load(nc.sync, kT[128 * i : 128 * (i + 1), :], i, f"k{i}")
                    for i in range(8)]
            qT_t = [x_load(nc.gpsimd, qT[128 * i : 128 * (i + 1), :], 8 + i,
                           f"q{i}") for i in range(8)]
            vT_t = [x_load(nc.sync, vT[128 * i : 128 * (i + 1), :], i, f"v{i}")
                    for i in range(8)]

            # Wk/Wv on scalar (ACT idle until the exp stream), Wq on gpsimd
            def wk_load(db, i):
                t = wk_pool.tile([128, 512], BF16, tag=f"wk{i}", name=f"wk{i}")
                nc.scalar.dma_start(
                    t[:], Wk[128 * i : 128 * (i + 1), 512 * db : 512 * (db + 1)]
                )
                return t

            wk_t = [wk_load(0, i) for i in range(8)]
            wk1_t = [wk_load(1, i) for i in range(8)]
            wv_t = []
            for i in range(8):
                t = wv_pool.tile([128, D], BF16, tag=f"wv{i}")
                nc.scalar.dma_start(t[:], Wv[128 * i : 128 * (i + 1), :])
                wv_t.append(t)

            # LN constants behind the weights on scalar (needed late)
            nc.scalar.dma_start(mbt[:], mb[:])
            nc.scalar.dma_start(eyet[:], env["eye"][:])
            nc.scalar.dma_start(gbt[:], gb[:])
            nc.scalar.dma_start(bbt[:], bb[:])

            def wq_load(db, i):
                t = wq_pool.tile([128, 512], BF16, tag=f"wq{i}", name=f"wq{i}")
                nc.gpsimd.dma_start(
                    t[:], Wq[128 * i : 128 * (i + 1), 512 * db : 512 * (db + 1)]
                )
                return t

            wq_t = [wq_load(0, i) for i in range(8)]

            # ---- K projection: khT_local[d, k]; CC pieces = k-col halves
            stagek_t = []
            for db in range(2):
                pss = [
                    pp_pool.tile([128, SS], F32, tag=f"pp{d}", name=f"pp{d}")
                    for d in range(4)
                ]
                for i in range(8):
                    wt = wk_t[i] if db == 0 else wk1_t[i]
                    for dsub in range(4):
                        nc.tensor.matmul(
                            pss[dsub][:], wt[:, 128 * dsub : 128 * (dsub + 1)],
                            kT_t[i][:], start=(i == 0), stop=(i == 7),
                        )
                for dsub in range(4):
                    dchunk = 4 * db + dsub
                    st = stagek_pool.tile([128, SS], BF16, tag=f"sk{dchunk}")
                    stagek_t.append(st)
                    nc.vector.tensor_copy(st[:], pss[dsub][:])
                    nc.gpsimd.dma_start(ck_in_v(0, dchunk), st[:, 0:HS])
                    nc.gpsimd.dma_start(ck_in_v(1, dchunk), st[:, HS:SS])
            # (K AllGathers are triggered after cv0's: chain order is
            # wu, V-half0, K-half0, K-half1, V-half1 so the first V piece
            # lands before the first remote PV needs it)

            # ---- Q projection ------------------------------------------
            for db in range(2):
                if db == 1:
                    wq_t = [wq_load(1, i) for i in range(8)]
                pss = [
                    pp_pool.tile([128, SS], F32, tag=f"pp{d}", name=f"pp{d}")
                    for d in range(4)
                ]
                for i in range(8):
                    for dsub in range(4):
                        nc.tensor.matmul(
                            pss[dsub][:],
                            wq_t[i][:, 128 * dsub : 128 * (dsub + 1)],
                            qT_t[i][:], start=(i == 0), stop=(i == 7),
                        )
                for dsub in range(4):
                    dchunk = 4 * db + dsub
                    qsubs = qhT_tiles[dchunk]
                    nc.vector.tensor_copy(qsubs[0][0:64, :], pss[dsub][0:64, :])
                    nc.vector.tensor_copy(
                        qsubs[1][64:128, :], pss[dsub][64:128, :]
                    )

            # pp_pool (K/Q projection PSUM) closes here so the local-pass
            # score PSUM fits alongside the V projection's ppv
            pp_stack.close()

            # ---- V projection, s-group sg: k-subs {2sg, 2sg+1} ---------
            stagev_t = [
                stagev_pool.tile([128, VW], F8, tag=f"sv{s}", name=f"sv{s}")
                for s in range(4)
            ]

            def v_proj(sg):
                for half in range(2):  # hd half: heads 8*half..
                    pvs = [
                        ppv_pool.tile(
                            [128, 512], F32, tag=f"pv{si}",
                            name=f"pv{2 * sg + si}_{half}",
                        )
                        for si in range(2)
                    ]
                    for i in range(8):
                        wm = wv_t[i][:, 512 * half : 512 * (half + 1)]
                        for si in range(2):
                            s = 2 * sg + si
                            nc.tensor.matmul(
                                pvs[si][:], vT_t[i][:, 128 * s : 128 * (s + 1)],
                                wm, start=(i == 0), stop=(i == 7),
                            )
                    for si in range(2):
                        s = 2 * sg + si
                        std = stagev_t[s][:].rearrange("p (h e) -> p h e", e=65)
                        nc.vector.tensor_copy(
                            std[:, 8 * half : 8 * (half + 1), 0:64],
                            pvs[si][:].rearrange("p (h e) -> p h e", e=64),
                        )
                for si in range(2):
                    s = 2 * sg + si
                    std = stagev_t[s][:].rearrange("p (h e) -> p h e", e=65)
                    nc.vector.memset(std[:, :, 64], 1.0)
                    nc.gpsimd.dma_start(cv_in_v(s), stagev_t[s][:])

            v_proj(0)
            ag(ck_in[0], ck_out[0])  # mesh 2: K k-cols 0:256
            ag(cv_in[0], cv_out[0])  # mesh 3: V k-subs 0,1
            ag(ck_in[1], ck_out[1])  # mesh 4: K k-cols 256:512

            # ================ Phase B: attention ========================
            # (opened while phase-A pools are live: the local pass halves
            # interleave with the V sg1 projection)
            pol_t = {}
            pol_gen = {}
            with contextlib.ExitStack() as stack_b:
                epb = stack_b.enter_context
                ps_pool = epb(tc.tile_pool(name="ps", bufs=2, space="PSUM"))
                po_pool = epb(tc.tile_pool(name="po", bufs=1, space="PSUM"))

                def score_pv(hp, kstat2, vstat2, bias, pos, first, last):
                    # both subs of one chunk-pair: 4 score MMs, 2 exps,
                    # 4 PV MMs, grouped by PE tile shape (all 128x128)
                    pss, pts = [], []
                    for sub in range(2):
                        qmov = qhT_tiles[hp][sub]
                        ps = ps_pool.tile([128, 2 * SS], F32, tag="ps", name="ps")
                        pss.append(ps)
                        for u in range(2):
                            nc.tensor.matmul(
                                ps[:, SS * u : SS * (u + 1)], kstat2[u], qmov,
                                start=True, stop=True,
                            )
                    for sub in range(2):
                        pt = pt_pool.tile([128, 2 * SS], F8, tag="pt", name="pt")
                        pts.append(pt)
                        if bias is None:
                            nc.scalar.activation(
                                pts[sub][:], pss[sub][:],
                                mybir.ActivationFunctionType.Exp, scale=0.125,
                            )
                        else:
                            nc.scalar.activation(
                                pts[sub][:], pss[sub][:],
                                mybir.ActivationFunctionType.Exp, scale=0.125,
                                bias=bias,
                            )
                    for sub in range(2):
                        h = 2 * hp + sub
                        for u in range(2):
                            nc.tensor.matmul(
                                pos[sub][:], vstat2[u][:, 65 * h : 65 * h + 65],
                                pts[sub][:, SS * u : SS * (u + 1)],
                                start=(first and u == 0), stop=(last and u == 1),
                            )

                def park(hp, sub, pos):
                    # partial -> SBUF f16; generations alternate between
                    # two tile tags, accumulating in place
                    prev = pol_t.get((hp, sub))
                    gen = pol_gen.get((hp, sub), 0)
                    ab = "AB"[gen % 2]
                    pl = pol_pool.tile(
                        [65, SS], F16, tag=f"pol{ab}{hp}_{sub}",
                        name=f"pol{ab}{hp}_{sub}",
                    )
                    if prev is None:
                        nc.vector.tensor_copy(pl[:], pos[:])
                    else:
                        nc.vector.tensor_add(pl[:], pos[:], prev[:])
                    pol_t[(hp, sub)] = pl
                    pol_gen[(hp, sub)] = gen + 1

                outT_tiles = []
                for i in range(8):
                    oT = outT_pool.tile([128, SS], BF16, tag=f"oT{i}")
                    outT_tiles.append(oT)

                def finish(hp, sub, pos):
                    # combine with parked partials, normalize, write outT
                    pl = pol_t[(hp, sub)]
                    tot = dn_pool.tile(
                        [65, SS], F32, tag=f"tot{sub}", name=f"tot{sub}"
                    )
                    nc.vector.tensor_add(tot[:], pos[:], pl[:])
                    rec = dn_pool.tile(
                        [1, SS], F32, tag=f"rec{sub}", name=f"rec{sub}"
                    )
                    nc.vector.tensor_copy(rec[:], tot[64:65, :])
                    rc2 = dn_pool.tile(
                        [1, SS], F32, tag=f"rc2{sub}", name=f"rc2{sub}"
                    )
                    nc.vector.reciprocal_approx_fast(rc2[:], rec[:])
                    rb = dn_pool.tile([64, SS], F32, tag=f"rb{sub}",
                                      name=f"rb{sub}")
                    nc.gpsimd.partition_broadcast(rb[:], rc2[:])
                    nc.vector.tensor_mul(
                        outT_tiles[hp][64 * sub : 64 * sub + 64, :],
                        tot[0:64, :], rb[:],
                    )

                def local_pass(sg):
                    # own-chunk attention over k-cols [256*sg, 256*sg+256)
                    for hp in range(H // 2):
                        pos = [
                            po_pool.tile(
                                [65, SS], F32, tag=f"po{s}", name=f"po{s}"
                            )
                            for s in range(2)
                        ]
                        kst = [
                            stagek_t[hp][:, 128 * (2 * sg + u) :
                                          128 * (2 * sg + u) + 128]
                            for u in range(2)
                        ]
                        vst = [stagev_t[2 * sg + u][:] for u in range(2)]
                        score_pv(hp, kst, vst, None, pos, True, True)
                        for sub in range(2):
                            park(hp, sub, pos[sub])

                # local attention on k-subs 0,1 overlaps the V sg1 MMs
                local_pass(0)
                v_proj(1)
                ag(cv_in[1], cv_out[1])  # mesh 5: V k-subs 2,3
                local_pass(1)

            stack_a.close()  # phase-A SBUF freed for wfc/resq
            with contextlib.ExitStack() as stack_b:
                epb = stack_b.enter_context
                wfc_pool = epb(tc.tile_pool(name="wfc", bufs=1))
                resq_pool = epb(tc.tile_pool(name="resq", bufs=1))
                psb_stack = contextlib.ExitStack()
                ps_pool = psb_stack.enter_context(
                    tc.tile_pool(name="ps", bufs=3, space="PSUM")
                )
                po_pool = psb_stack.enter_context(
                    tc.tile_pool(name="po", bufs=1, space="PSUM")
                )
                # the gathered passes only touch the 3 REMOTE chunks:
                # chunk index c = (rank + j) % 4 for j=1..3, addressed via
                # sync-engine registers loaded from the per-core rank input
                rrank = nc.sync.alloc_register("rrank")
                nc.sync.reg_load(rrank, env["rk"][0:1, 0:1])
                rk8 = []  # c*1024: ck_out row offset of remote chunk j
                rv2 = []  # c*256: cv_out row offset of remote chunk j
                for j in range(1, 4):
                    r8 = nc.sync.alloc_register(f"rk8_{j}")
                    nc.sync.reg_alu(r8, rrank, j, mybir.AluOpType.add)
                    nc.sync.reg_alu(r8, r8, 3, mybir.AluOpType.bitwise_and)
                    r2 = nc.sync.alloc_register(f"rv2_{j}")
                    nc.sync.reg_alu(r2, r8, 256, mybir.AluOpType.mult)
                    nc.sync.reg_alu(r8, r8, 1024, mybir.AluOpType.mult)
                    rk8.append(nc.sync.snap(r8))
                    rv2.append(nc.sync.snap(r2))

                vh_t = {}

                def vh_load(eng, j, s):
                    t = vh_pool.tile(
                        [128, VW], F8, tag=f"vh{j}_{s}", name=f"vh{j}_{s}"
                    )
                    eng.dma_start(t[:], cv_out_v(rv2[j - 1], s))
                    vh_t[(j, s)] = t

                def khr_load(p, hp):
                    ts = {}
                    for j in range(1, 4):
                        t = khr_pool.tile(
                            [128, HS], BF16, tag=f"khc{j}", name=f"khc{j}"
                        )
                        nc.sync.dma_start(t[:], ck_out_v(p, rk8[j - 1], hp))
                        ts[j] = t
                    return ts

                # sync-queue load order tracks the mesh chain: kh for the
                # first head-pairs (mesh k0), then pass-0 vh (mesh v0),
                # then the rest -- so no load head-of-line blocks another
                # that could already run, and PV stalls stay tiny.
                khs = [khr_load(0, hp) for hp in range(4)]
                for j in range(1, 4):  # pass-0 vh tiles (k-subs 0,1)
                    for s in range(2):
                        vh_load(nc.sync, j, s)
                khs += [khr_load(0, hp) for hp in range(4, 8)]
                for j in range(1, 4):  # pass-1 vh tiles (k-subs 2,3)
                    for s in range(2, 4):
                        vh_load(nc.sync, j, s)

                # wfc / qn prefetch on gpsimd behind the vh loads
                wfc_t = []
                for i in range(8):
                    t = wfc_pool.tile([128, D], BF16, tag=f"wfc{i}")
                    nc.gpsimd.dma_start(t[:], Wfc[128 * i : 128 * (i + 1), :])
                    wfc_t.append(t)
                qn_t = []
                for qs in range(4):
                    t = resq_pool.tile([128, D], BF16, tag=f"qn{qs}")
                    nc.gpsimd.dma_start(t[:], qn[128 * qs : 128 * (qs + 1), :])
                    qn_t.append(t)

                for p in range(2):
                    for hp in range(H // 2):
                        kh_t = khs[hp] if p == 0 else khr_load(p, hp)

                        pos = [
                            po_pool.tile([65, SS], F32, tag=f"po{s}",
                                         name=f"po{s}")
                            for s in range(2)
                        ]
                        for j in range(1, 4):
                            kst = [kh_t[j][:, 128 * u : 128 * (u + 1)]
                                   for u in range(2)]
                            vst = [vh_t[(j, 2 * p + u)][:] for u in range(2)]
                            score_pv(
                                hp, kst, vst, None,
                                pos, first=(j == 1), last=(j == 3),
                            )
                        for sub in range(2):
                            if p == 0:
                                park(hp, sub, pos[sub])
                            else:
                                finish(hp, sub, pos[sub])

                # ============ Phase C: fc + residual + LayerNorm ========
                psb_stack.close()  # attention PSUM freed for the fc psums
                with contextlib.ExitStack() as stack_c:
                    epc = stack_c.enter_context
                    pfc_pool = epc(tc.tile_pool(name="pfc", bufs=1, space="PSUM"))
                    lns_pool = epc(tc.tile_pool(name="lns", bufs=1))
                    lnsc_pool = epc(tc.tile_pool(name="lnsc", bufs=1))
                    # stage-major emission: all four q-subtiles advance
                    # together so the per-subtile serial chain (fc -> add ->
                    # mean -> var -> rstd -> scale) pipelines across ACT/DVE
                    pf_l, x_l, nmu_l, rstd_l = [], [], [], []

                    def fc_qs(qs):
                        # fc + residual: the residual rows ride into the
                        # same PSUM accumulation as an identity matmul --
                        # the PE is idle here and it deletes a serial
                        # [128,1024] DVE add per q-subtile
                        pf = pfc_pool.tile([128, D], F32, tag=f"pf{qs}")
                        for i in range(8):
                            stat = outT_tiles[i][:, 128 * qs : 128 * (qs + 1)]
                            nc.tensor.matmul(
                                pf[:, 0:512], stat, wfc_t[i][:, 0:512],
                                start=(i == 0), stop=False,
                            )
                            nc.tensor.matmul(
                                pf[:, 512:1024], stat, wfc_t[i][:, 512:1024],
                                start=(i == 0), stop=False,
                            )
                        for h2 in range(2):
                            nc.tensor.matmul(
                                pf[:, 512 * h2 : 512 * (h2 + 1)], eyet[:],
                                qn_t[qs][:, 512 * h2 : 512 * (h2 + 1)],
                                start=False, stop=True,
                            )
                        pf_l.append(pf)
                        x_l.append(pf)

                    # q-subtile 0's chain leads so the LN ACT stream starts
                    # while the other three fc groups still run on the PE
                    for qs in range(4):
                        fc_qs(qs)
                    for qs in range(4):
                        msum = lnsc_pool.tile([128, 1], F32, tag=f"msum{qs}")
                        nc.vector.reduce_sum(
                            out=msum[:], in_=x_l[qs][:], axis=mybir.AxisListType.X
                        )
                        nmu = lnsc_pool.tile([128, 1], F32, tag=f"nmu{qs}")
                        nc.scalar.activation(
                            nmu[:], msum[:], mybir.ActivationFunctionType.Copy,
                            scale=-1.0 / D,
                        )
                        nmu_l.append(nmu)
                    for qs in range(4):
                        sq = lns_pool.tile([128, D], F32, tag="t", name="sq")
                        vsum = lnsc_pool.tile([128, 1], F32, tag=f"vsum{qs}")
                        nc.scalar.activation(
                            sq[:], x_l[qs][:],
                            mybir.ActivationFunctionType.Square,
                            bias=nmu_l[qs][:], accum_out=vsum[:],
                        )
                        std = lnsc_pool.tile([128, 1], F32, tag=f"std{qs}")
                        nc.scalar.activation(
                            std[:], vsum[:],
                            mybir.ActivationFunctionType.Sqrt,
                            scale=1.0 / D, bias=epst[:],
                        )
                        rstd = lnsc_pool.tile([128, 1], F32, tag=f"rstd{qs}")
                        nc.vector.reciprocal(rstd[:], std[:])
                        rstd_l.append(rstd)
                    for qs in range(4):
                        xn = lns_pool.tile([128, D], F32, tag="t", name="xn")
                        nc.vector.tensor_scalar(
                            out=xn[:], in0=x_l[qs][:], scalar1=nmu_l[qs][:],
                            scalar2=rstd_l[qs][:],
                            op0=mybir.AluOpType.add, op1=mybir.AluOpType.mult,
                        )
                        xg = lns_pool.tile([128, D], F32, tag="g", name="xg")
                        nc.vector.tensor_mul(xg[:], xn[:], gbt[:])
                        xb = lns_pool.tile([128, D], F32, tag="b", name="xb")
                        nc.vector.tensor_add(xb[:], xg[:], bbt[:])
                        nc.sync.dma_start(
                            out[128 * qs : 128 * (qs + 1), :], xb[:]
                        )

_NC_CACHE = None


def kernel(q, k, v, Wq, Wk, Wv, Wfc, bfc, gamma, beta):
    global _NC_CACHE
    if _NC_CACHE is None:
        _NC_CACHE = build_kernel()
    nc = _NC_CACHE

    bf16 = ml_dtypes.bfloat16
    q = np.asarray(q, dtype=np.float32)
    k = np.asarray(k, dtype=np.float32)
    v = np.asarray(v, dtype=np.float32)
    Wq = np.ascontiguousarray(np.asarray(Wq, dtype=np.float32).astype(bf16))
    Wk = np.ascontiguousarray(np.asarray(Wk, dtype=np.float32).astype(bf16))
    Wv = np.ascontiguousarray(np.asarray(Wv, dtype=np.float32).astype(bf16))
    Wfc = np.ascontiguousarray(np.asarray(Wfc, dtype=np.float32).astype(bf16))
    bfc = np.asarray(bfc, dtype=np.float32)
    gamma = np.asarray(gamma, dtype=np.float32)
    beta = np.asarray(beta, dtype=np.float32)

    gb = np.ascontiguousarray(np.broadcast_to(gamma, (128, D)))
    eye = np.ascontiguousarray(np.eye(128, dtype=np.float32).astype(bf16))
    bb = np.ascontiguousarray(np.broadcast_to(beta, (128, D)))

    in_maps = []
    for c in range(N_CORES):
        b, r0 = c // 4, (c % 4) * SS
        qs = q[b, r0 : r0 + SS]
        ks = k[b, r0 : r0 + SS]
        vs = v[b, r0 : r0 + SS]
        mbm = np.zeros((128, 4), np.float32)
        mbm[:, c % 4] = -30000.0
        in_maps.append(
            {
                "rk": np.array([[c % 4, 0, 0, 0]], dtype=np.int32),
                "eye": eye,
                "qT": np.ascontiguousarray(qs.T.astype(bf16)),
                "kT": np.ascontiguousarray(ks.T.astype(bf16)),
                "vT": np.ascontiguousarray(vs.T.astype(bf16)),
                "qn": np.ascontiguousarray((qs + bfc).astype(bf16)),
                "Wq": Wq, "Wk": Wk, "Wv": Wv, "Wfc": Wfc,
                "gb": gb, "bb": bb, "mb": mbm,
            }
        )

    global _last_in_maps
    _last_in_maps = in_maps
    res = run_bass_kernel_spmd(nc, in_maps, list(range(N_CORES)))
    out = np.empty((B, S, D), dtype=np.float32)
    for c in range(N_CORES):
        b, r0 = c // 4, (c % 4) * SS
        out[b, r0 : r0 + SS] = res.results[c]["out"]
    return out


# revision 50
# speedup vs baseline: 1.0874x; 1.0806x over previous
"""Distributed MultiHeadAttention (+residual, +LayerNorm) Trainium2 kernel.

Problem: B=2, S=2048, D_MODEL=1024, N_HEAD=16, D_K=D_V=64, eps=1e-6.
  qh = q@Wq, kh = k@Wk, vh = v@Wv  (per head)
  attn = softmax(qh·kh^T / 8)
  out = (attn@vh) @ Wfc + bfc + q  -> LayerNorm(gamma, beta)

Sharding: 8 cores; core c owns 512 q-rows of batch c//4 (sequence shard).
Each core projects K/V for its own 512 rows; AllGathers over each 4-core
batch group materialize the full-batch K^T/V; attention, fc and
LayerNorm are then fully local.

v2 schedule (from trace analysis of the baseline):
  - the serial CC core boots ~55us into the kernel and then executes
    meshes back-to-back; compute must never wait on a piece it needs
    early.  The 5 AllGathers are trigger-chained (nosync deps) in
    consumption order: wu warmup, K-half0, V-half0, K-half1, V-half1.
  - the exp stream on ACT is the critical resource (160 exps of
    [128,1024] ~= 184us).  It is fed as early as possible: projections
    run K -> Q -> V(k-subs 0,1), then the local attention pass over own
    k-cols 0:255 starts while V(k-subs 2,3) projects, then local pass
    over own k-cols 256:511.  The remote passes chew the gathered
    pieces the moment they land.
  - DMA queues: sync = kT/vT + gathered kh + vh(s01) + out; scalar =
    Wk/Wv + constants; gpsimd = qT/Wq + collective staging/triggers +
    vh(s23) + Wfc/qn.
Own-chunk contributions are counted once: the gathered passes mask the
core's own chunk to exp()=0 via a per-core -30000 bias input.
"""

import sys

sys.path.insert(0, "/opt/trn_rl_repo")

import ml_dtypes
import numpy as np

import contextlib

import concourse.bass as bass
import concourse.tile as tile
from concourse import bacc, mybir
from concourse.bass import _add_dep_helper
from concourse.bass_utils import run_bass_kernel_spmd

N_CORES = 8
B = 2
S = 2048
D = 1024  # d_model
H = 16  # heads
DK = 64  # head dim
SS = S // 4  # 512 q-rows per core
HS = SS // 2  # 256 k-columns per K collective piece
LN_EPS = 1e-6
F32 = mybir.dt.float32
F16 = mybir.dt.float16
BF16 = mybir.dt.bfloat16
F8 = mybir.dt.float8e4

VW = H * 65  # vh row width with per-head ones column baked in


def build_kernel():
    nc = bacc.Bacc()

    qT = nc.dram_tensor("qT", [D, SS], BF16, kind="ExternalInput")
    kT = nc.dram_tensor("kT", [D, SS], BF16, kind="ExternalInput")
    vT = nc.dram_tensor("vT", [D, SS], BF16, kind="ExternalInput")
    qn = nc.dram_tensor("qn", [SS, D], F32, kind="ExternalInput")  # residual + bfc
    Wq = nc.dram_tensor("Wq", [D, D], BF16, kind="ExternalInput")
    Wk = nc.dram_tensor("Wk", [D, D], BF16, kind="ExternalInput")
    Wv = nc.dram_tensor("Wv", [D, D], BF16, kind="ExternalInput")
    Wfc = nc.dram_tensor("Wfc", [D, D], BF16, kind="ExternalInput")
    gb = nc.dram_tensor("gb", [128, D], F32, kind="ExternalInput")  # gamma bcast
    bb = nc.dram_tensor("bb", [128, D], F32, kind="ExternalInput")  # beta bcast
    mb = nc.dram_tensor("mb", [128, 4], F32, kind="ExternalInput")  # own-chunk mask
    out = nc.dram_tensor("out", [SS, D], F32, kind="ExternalOutput")

    # collective staging, split into consumption-ordered pieces.
    # K pieces are k-column halves (all head-pairs usable per piece).
    ck_in = [nc.dram_tensor(f"ck{p}_in", [8 * 128, HS], BF16) for p in range(2)]
    ck_out = [nc.dram_tensor(f"ck{p}_out", [32 * 128, HS], BF16) for p in range(2)]
    cv_in = [nc.dram_tensor(f"cv{p}_in", [2 * 128, VW], F8) for p in range(2)]
    cv_out = [nc.dram_tensor(f"cv{p}_out", [8 * 128, VW], F8) for p in range(2)]
    rk = nc.dram_tensor("rk", [1, 4], mybir.dt.int32, kind="ExternalInput")
    wu_in = nc.dram_tensor("wu_in", [128], BF16)
    wu_out = nc.dram_tensor("wu_out", [512], BF16)

    def ck_in_v(p, i):  # k-half p, khT d-chunk i (0..7) -> [128, 256] view
        return ck_in[p][128 * i : 128 * (i + 1), :]

    def ck_out_v(p, rc8, i):  # k-half p, chunk-row reg, d-chunk i -> [128, 256]
        # rc8 holds c*1024 (the gathered chunk's row offset)
        return ck_out[p][128 * i :][bass.ds(rc8, 128), :]

    def cv_in_v(s):  # vh k-sub s (0..3) -> [128, 1040] staging view
        p, l = divmod(s, 2)
        return cv_in[p][128 * l : 128 * (l + 1), :]

    def cv_out_v(rc2, s):  # chunk-row reg (c*256), k-sub s -> [128, 1040]
        p, l = divmod(s, 2)
        return cv_out[p][128 * l :][bass.ds(rc2, 128), :]

    env = dict(locals())
    with tile.TileContext(nc) as tc:
        _build_body(nc, tc, env)
    nc.compile()
    return nc


def _build_body(nc, tc, env):
    qT = env["qT"]; kT = env["kT"]; vT = env["vT"]; qn = env["qn"]
    Wq = env["Wq"]; Wk = env["Wk"]; Wv = env["Wv"]; Wfc = env["Wfc"]
    gb = env["gb"]; bb = env["bb"]; mb = env["mb"]; out = env["out"]
    ck_in = env["ck_in"]; ck_out = env["ck_out"]
    cv_in = env["cv_in"]; cv_out = env["cv_out"]
    ck_in_v = env["ck_in_v"]; ck_out_v = env["ck_out_v"]
    cv_in_v = env["cv_in_v"]; cv_out_v = env["cv_out_v"]
    RG = [[0, 1, 2, 3], [4, 5, 6, 7]]

    cc_chain = []

    def ag(tin, tout):
        cc = nc.gpsimd.collective_compute(
            "AllGather", mybir.AluOpType.bypass, replica_groups=RG,
            ins=[tin[:]], outs=[tout[:]],
        )
        if cc_chain:
            _add_dep_helper(
                cc.ins, cc_chain[-1].ins, sync=False, reason="cc issue order"
            )
        cc_chain.append(cc)

    with contextlib.ExitStack() as stack:
        ep = stack.enter_context
        # persistent SBUF (fresh addresses, no WAR with phase-A pools)
        qhT_pool = ep(tc.tile_pool(name="qhT", bufs=1))
        stagek_pool = ep(tc.tile_pool(name="stagek", bufs=1))
        stagev_pool = ep(tc.tile_pool(name="stagev", bufs=1))
        outT_pool = ep(tc.tile_pool(name="outT", bufs=1))
        vh_pool = ep(tc.tile_pool(name="vh", bufs=1))
        khr_pool = ep(tc.tile_pool(name="khr", bufs=8))
        pt_pool = ep(tc.tile_pool(name="pt", bufs=26))
        pol_pool = ep(tc.tile_pool(name="pol", bufs=1))
        cst_pool = ep(tc.tile_pool(name="cst", bufs=1))
        dn_pool = ep(tc.tile_pool(name="dn", bufs=1))
        # collective warmup: tiny AllGather triggered before anything else
        # (the CC core takes ~55us to boot; this starts that clock)
        ag(env["wu_in"], env["wu_out"])

        # ---- constants (scalar queue, behind nothing critical yet) ------
        mbt = cst_pool.tile([128, 4], F32, tag="mb")
        gbt = cst_pool.tile([128, D], F32, tag="gb")
        bbt = cst_pool.tile([128, D], F32, tag="bb")
        epst = cst_pool.tile([128, 1], F32, tag="eps")
        nc.vector.memset(epst[:], LN_EPS)

        # zero-padded per-sub q moving tiles (zeros written now, the live
        # halves copied in after the Q projection) -> every attention
        # matmul runs with a uniform full-128-row stationary tile config
        qhT_tiles = []
        for dchunk in range(8):
            qsubs = [
                qhT_pool.tile(
                    [128, SS], BF16, tag=f"qh{dchunk}_{sub}",
                    name=f"qh{dchunk}_{sub}",
                )
                for sub in range(2)
            ]
            nc.vector.memset(qsubs[0][64:128, :], 0.0)
            nc.vector.memset(qsubs[1][0:64, :], 0.0)
            qhT_tiles.append(qsubs)

        # ================ Phase A: QKV projections ======================
        stack_a = contextlib.ExitStack()
        if True:
            epa = stack_a.enter_context
            xin_pool = epa(tc.tile_pool(name="xin", bufs=1))
            wk_pool = epa(tc.tile_pool(name="wk", bufs=1))
            wv_pool = epa(tc.tile_pool(name="wv", bufs=1))
            wq_pool = epa(tc.tile_pool(name="wq", bufs=1))
            ppv_pool = epa(tc.tile_pool(name="ppv", bufs=1, space="PSUM"))
            pp_stack = contextlib.ExitStack()
            pp_pool = pp_stack.enter_context(
                tc.tile_pool(name="pp", bufs=1, space="PSUM")
            )
            # kT/vT on sync, qT on gpsimd (kT reuses slots with nothing)
            def x_load(eng, srct, slot, tag):
                t = xin_pool.tile([128, SS], BF16, tag=f"x{slot}", name=f"x{tag}")
                eng.dma_start(t[:], srct)
                return t

            kT_t = [x_load(nc.sync, kT[128 * i : 128 * (i + 1), :], i, f"k{i}")
                    for i in range(8)]
            qT_t = [x_load(nc.gpsimd, qT[128 * i : 128 * (i + 1), :], 8 + i,
                           f"q{i}") for i in range(8)]
            vT_t = [x_load(nc.sync, vT[128 * i : 128 * (i + 1), :], i, f"v{i}")
                    for i in range(8)]

            # Wk/Wv on scalar (ACT idle until the exp stream), Wq on gpsimd
            def wk_load(db, i):
                t = wk_pool.tile([128, 512], BF16, tag=f"wk{i}", name=f"wk{i}")
                nc.scalar.dma_start(
                    t[:], Wk[128 * i : 128 * (i + 1), 512 * db : 512 * (db + 1)]
                )
                return t

            wk_t = [wk_load(0, i) for i in range(8)]
            wk1_t = [wk_load(1, i) for i in range(8)]
            wv_t = []
            for i in range(8):
                t = wv_pool.tile([128, D], BF16, tag=f"wv{i}")
                nc.scalar.dma_start(t[:], Wv[128 * i : 128 * (i + 1), :])
                wv_t.append(t)

            # LN constants behind the weights on scalar (needed late)
            nc.scalar.dma_start(mbt[:], mb[:])
            nc.scalar.dma_start(gbt[:], gb[:])
            nc.scalar.dma_start(bbt[:], bb[:])

            def wq_load(db, i):
                t = wq_pool.tile([128, 512], BF16, tag=f"wq{i}", name=f"wq{i}")
                nc.gpsimd.dma_start(
                    t[:], Wq[128 * i : 128 * (i + 1), 512 * db : 512 * (db + 1)]
                )
                return t

            wq_t = [wq_load(0, i) for i in range(8)]

            # ---- K projection: khT_local[d, k]; CC pieces = k-col halves
            stagek_t = []
            for db in range(2):
                pss = [
                    pp_pool.tile([128, SS], F32, tag=f"pp{d}", name=f"pp{d}")
                    for d in range(4)
                ]
                for i in range(8):
                    wt = wk_t[i] if db == 0 else wk1_t[i]
                    for dsub in range(4):
                        nc.tensor.matmul(
                            pss[dsub][:], wt[:, 128 * dsub : 128 * (dsub + 1)],
                            kT_t[i][:], start=(i == 0), stop=(i == 7),
                        )
                for dsub in range(4):
                    dchunk = 4 * db + dsub
                    st = stagek_pool.tile([128, SS], BF16, tag=f"sk{dchunk}")
                    stagek_t.append(st)
                    nc.vector.tensor_copy(st[:], pss[dsub][:])
                    nc.gpsimd.dma_start(ck_in_v(0, dchunk), st[:, 0:HS])
                    nc.gpsimd.dma_start(ck_in_v(1, dchunk), st[:, HS:SS])
            # (K AllGathers are triggered after cv0's: chain order is
            # wu, V-half0, K-half0, K-half1, V-half1 so the first V piece
            # lands before the first remote PV needs it)

            # ---- Q projection ------------------------------------------
            for db in range(2):
                if db == 1:
                    wq_t = [wq_load(1, i) for i in range(8)]
                pss = [
                    pp_pool.tile([128, SS], F32, tag=f"pp{d}", name=f"pp{d}")
                    for d in range(4)
                ]
                for i in range(8):
                    for dsub in range(4):
                        nc.tensor.matmul(
                            pss[dsub][:],
                            wq_t[i][:, 128 * dsub : 128 * (dsub + 1)],
                            qT_t[i][:], start=(i == 0), stop=(i == 7),
                        )
                for dsub in range(4):
                    dchunk = 4 * db + dsub
                    qsubs = qhT_tiles[dchunk]
                    nc.vector.tensor_copy(qsubs[0][0:64, :], pss[dsub][0:64, :])
                    nc.vector.tensor_copy(
                        qsubs[1][64:128, :], pss[dsub][64:128, :]
                    )

            # pp_pool (K/Q projection PSUM) closes here so the local-pass
            # score PSUM fits alongside the V projection's ppv
            pp_stack.close()

            # ---- V projection, s-group sg: k-subs {2sg, 2sg+1} ---------
            stagev_t = [
                stagev_pool.tile([128, VW], F8, tag=f"sv{s}", name=f"sv{s}")
                for s in range(4)
            ]

            def v_proj(sg):
                for half in range(2):  # hd half: heads 8*half..
                    pvs = [
                        ppv_pool.tile(
                            [128, 512], F32, tag=f"pv{si}",
                            name=f"pv{2 * sg + si}_{half}",
                        )
                        for si in range(2)
                    ]
                    for i in range(8):
                        wm = wv_t[i][:, 512 * half : 512 * (half + 1)]
                        for si in range(2):
                            s = 2 * sg + si
                            nc.tensor.matmul(
                                pvs[si][:], vT_t[i][:, 128 * s : 128 * (s + 1)],
                                wm, start=(i == 0), stop=(i == 7),
                            )
                    for si in range(2):
                        s = 2 * sg + si
                        std = stagev_t[s][:].rearrange("p (h e) -> p h e", e=65)
                        nc.vector.tensor_copy(
                            std[:, 8 * half : 8 * (half + 1), 0:64],
                            pvs[si][:].rearrange("p (h e) -> p h e", e=64),
                        )
                for si in range(2):
                    s = 2 * sg + si
                    std = stagev_t[s][:].rearrange("p (h e) -> p h e", e=65)
                    nc.vector.memset(std[:, :, 64], 1.0)
                    nc.gpsimd.dma_start(cv_in_v(s), stagev_t[s][:])

            v_proj(0)
            ag(ck_in[0], ck_out[0])  # mesh 2: K k-cols 0:256
            ag(cv_in[0], cv_out[0])  # mesh 3: V k-subs 0,1
            ag(ck_in[1], ck_out[1])  # mesh 4: K k-cols 256:512

            # ================ Phase B: attention ========================
            # (opened while phase-A pools are live: the local pass halves
            # interleave with the V sg1 projection)
            pol_t = {}
            pol_gen = {}
            with contextlib.ExitStack() as stack_b:
                epb = stack_b.enter_context
                ps_pool = epb(tc.tile_pool(name="ps", bufs=2, space="PSUM"))
                po_pool = epb(tc.tile_pool(name="po", bufs=1, space="PSUM"))

                def score_pv(hp, kstat2, vstat2, bias, pos, first, last):
                    # both subs of one chunk-pair: 4 score MMs, 2 exps,
                    # 4 PV MMs, grouped by PE tile shape (all 128x128)
                    pss, pts = [], []
                    for sub in range(2):
                        qmov = qhT_tiles[hp][sub]
                        ps = ps_pool.tile([128, 2 * SS], F32, tag="ps", name="ps")
                        pss.append(ps)
                        for u in range(2):
                            nc.tensor.matmul(
                                ps[:, SS * u : SS * (u + 1)], kstat2[u], qmov,
                                start=True, stop=True,
                            )
                    for sub in range(2):
                        pt = pt_pool.tile([128, 2 * SS], F8, tag="pt", name="pt")
                        pts.append(pt)
                        if bias is None:
                            nc.scalar.activation(
                                pts[sub][:], pss[sub][:],
                                mybir.ActivationFunctionType.Exp, scale=0.125,
                            )
                        else:
                            nc.scalar.activation(
                                pts[sub][:], pss[sub][:],
                                mybir.ActivationFunctionType.Exp, scale=0.125,
                                bias=bias,
                            )
                    for sub in range(2):
                        h = 2 * hp + sub
                        for u in range(2):
                            nc.tensor.matmul(
                                pos[sub][:], vstat2[u][:, 65 * h : 65 * h + 65],
                                pts[sub][:, SS * u : SS * (u + 1)],
                                start=(first and u == 0), stop=(last and u == 1),
                            )

                def park(hp, sub, pos):
                    # partial -> SBUF f16; generations alternate between
                    # two tile tags, accumulating in place
                    prev = pol_t.get((hp, sub))
                    gen = pol_gen.get((hp, sub), 0)
                    ab = "AB"[gen % 2]
                    pl = pol_pool.tile(
                        [65, SS], F16, tag=f"pol{ab}{hp}_{sub}",
                        name=f"pol{ab}{hp}_{sub}",
                    )
                    if prev is None:
                        nc.vector.tensor_copy(pl[:], pos[:])
                    else:
                        nc.vector.tensor_add(pl[:], pos[:], prev[:])
                    pol_t[(hp, sub)] = pl
                    pol_gen[(hp, sub)] = gen + 1

                outT_tiles = []
                for i in range(8):
                    oT = outT_pool.tile([128, SS], BF16, tag=f"oT{i}")
                    outT_tiles.append(oT)

                def finish(hp, sub, pos):
                    # combine with parked partials, normalize, write outT
                    pl = pol_t[(hp, sub)]
                    tot = dn_pool.tile(
                        [65, SS], F32, tag=f"tot{sub}", name=f"tot{sub}"
                    )
                    nc.vector.tensor_add(tot[:], pos[:], pl[:])
                    rec = dn_pool.tile(
                        [1, SS], F32, tag=f"rec{sub}", name=f"rec{sub}"
                    )
                    nc.vector.tensor_copy(rec[:], tot[64:65, :])
                    rc2 = dn_pool.tile(
                        [1, SS], F32, tag=f"rc2{sub}", name=f"rc2{sub}"
                    )
                    nc.vector.reciprocal_approx_fast(rc2[:], rec[:])
                    rb = dn_pool.tile([64, SS], F32, tag=f"rb{sub}",
                                      name=f"rb{sub}")
                    nc.gpsimd.partition_broadcast(rb[:], rc2[:])
                    nc.vector.tensor_mul(
                        outT_tiles[hp][64 * sub : 64 * sub + 64, :],
                        tot[0:64, :], rb[:],
                    )

                def local_pass(sg):
                    # own-chunk attention over k-cols [256*sg, 256*sg+256)
                    for hp in range(H // 2):
                        pos = [
                            po_pool.tile(
                                [65, SS], F32, tag=f"po{s}", name=f"po{s}"
                            )
                            for s in range(2)
                        ]
                        kst = [
                            stagek_t[hp][:, 128 * (2 * sg + u) :
                                          128 * (2 * sg + u) + 128]
                            for u in range(2)
                        ]
                        vst = [stagev_t[2 * sg + u][:] for u in range(2)]
                        score_pv(hp, kst, vst, None, pos, True, True)
                        for sub in range(2):
                            park(hp, sub, pos[sub])

                # local attention on k-subs 0,1 overlaps the V sg1 MMs
                local_pass(0)
                v_proj(1)
                ag(cv_in[1], cv_out[1])  # mesh 5: V k-subs 2,3
                local_pass(1)

            stack_a.close()  # phase-A SBUF freed for wfc/resq
            with contextlib.ExitStack() as stack_b:
                epb = stack_b.enter_context
                wfc_pool = epb(tc.tile_pool(name="wfc", bufs=1))
                resq_pool = epb(tc.tile_pool(name="resq", bufs=1))
                psb_stack = contextlib.ExitStack()
                ps_pool = psb_stack.enter_context(
                    tc.tile_pool(name="ps", bufs=3, space="PSUM")
                )
                po_pool = psb_stack.enter_context(
                    tc.tile_pool(name="po", bufs=1, space="PSUM")
                )
                # the gathered passes only touch the 3 REMOTE chunks:
                # chunk index c = (rank + j) % 4 for j=1..3, addressed via
                # sync-engine registers loaded from the per-core rank input
                rrank = nc.sync.alloc_register("rrank")
                nc.sync.reg_load(rrank, env["rk"][0:1, 0:1])
                rk8 = []  # c*1024: ck_out row offset of remote chunk j
                rv2 = []  # c*256: cv_out row offset of remote chunk j
                for j in range(1, 4):
                    r8 = nc.sync.alloc_register(f"rk8_{j}")
                    nc.sync.reg_alu(r8, rrank, j, mybir.AluOpType.add)
                    nc.sync.reg_alu(r8, r8, 3, mybir.AluOpType.bitwise_and)
                    r2 = nc.sync.alloc_register(f"rv2_{j}")
                    nc.sync.reg_alu(r2, r8, 256, mybir.AluOpType.mult)
                    nc.sync.reg_alu(r8, r8, 1024, mybir.AluOpType.mult)
                    rk8.append(nc.sync.snap(r8))
                    rv2.append(nc.sync.snap(r2))

                vh_t = {}

                def vh_load(eng, j, s):
                    t = vh_pool.tile(
                        [128, VW], F8, tag=f"vh{j}_{s}", name=f"vh{j}_{s}"
                    )
                    eng.dma_start(t[:], cv_out_v(rv2[j - 1], s))
                    vh_t[(j, s)] = t

                def khr_load(p, hp):
                    ts = {}
                    for j in range(1, 4):
                        t = khr_pool.tile(
                            [128, HS], BF16, tag=f"khc{j}", name=f"khc{j}"
                        )
                        nc.sync.dma_start(t[:], ck_out_v(p, rk8[j - 1], hp))
                        ts[j] = t
                    return ts

                # sync-queue load order tracks the mesh chain: kh for the
                # first head-pairs (mesh k0), then pass-0 vh (mesh v0),
                # then the rest -- so no load head-of-line blocks another
                # that could already run, and PV stalls stay tiny.
                khs = [khr_load(0, hp) for hp in range(4)]
                for j in range(1, 4):  # pass-0 vh tiles (k-subs 0,1)
                    for s in range(2):
                        vh_load(nc.sync, j, s)
                khs += [khr_load(0, hp) for hp in range(4, 8)]
                for j in range(1, 4):  # pass-1 vh tiles (k-subs 2,3)
                    for s in range(2, 4):
                        vh_load(nc.sync, j, s)

                # wfc / qn prefetch on gpsimd behind the vh loads
                wfc_t = []
                for i in range(8):
                    t = wfc_pool.tile([128, D], BF16, tag=f"wfc{i}")
                    nc.gpsimd.dma_start(t[:], Wfc[128 * i : 128 * (i + 1), :])
                    wfc_t.append(t)
                qn_t = []
                for qs in range(4):
                    t = resq_pool.tile([128, D], F32, tag=f"qn{qs}")
                    nc.gpsimd.dma_start(t[:], qn[128 * qs : 128 * (qs + 1), :])
                    qn_t.append(t)

                for p in range(2):
                    for hp in range(H // 2):
                        kh_t = khs[hp] if p == 0 else khr_load(p, hp)

                        pos = [
                            po_pool.tile([65, SS], F32, tag=f"po{s}",
                                         name=f"po{s}")
                            for s in range(2)
                        ]
                        for j in range(1, 4):
                            kst = [kh_t[j][:, 128 * u : 128 * (u + 1)]
                                   for u in range(2)]
                            vst = [vh_t[(j, 2 * p + u)][:] for u in range(2)]
                            score_pv(
                                hp, kst, vst, None,
                                pos, first=(j == 1), last=(j == 3),
                            )
                        for sub in range(2):
                            if p == 0:
                                park(hp, sub, pos[sub])
                            else:
                                finish(hp, sub, pos[sub])

                # ============ Phase C: fc + residual + LayerNorm ========
                psb_stack.close()  # attention PSUM freed for the fc psums
                with contextlib.ExitStack() as stack_c:
                    epc = stack_c.enter_context
                    pfc_pool = epc(tc.tile_pool(name="pfc", bufs=1, space="PSUM"))
                    lns_pool = epc(tc.tile_pool(name="lns", bufs=1))
                    lnsc_pool = epc(tc.tile_pool(name="lnsc", bufs=1))
                    # stage-major emission: all four q-subtiles advance
                    # together so the per-subtile serial chain (fc -> add ->
                    # mean -> var -> rstd -> scale) pipelines across ACT/DVE
                    pf_l, x_l, nmu_l, rstd_l = [], [], [], []

                    def fc_qs(qs):
                        pf = pfc_pool.tile([128, D], F32, tag=f"pf{qs}")
                        for i in range(8):
                            stat = outT_tiles[i][:, 128 * qs : 128 * (qs + 1)]
                            nc.tensor.matmul(
                                pf[:, 0:512], stat, wfc_t[i][:, 0:512],
                                start=(i == 0), stop=(i == 7),
                            )
                            nc.tensor.matmul(
                                pf[:, 512:1024], stat, wfc_t[i][:, 512:1024],
                                start=(i == 0), stop=(i == 7),
                            )
                        pf_l.append(pf)

                    def x_qs(qs):
                        x = lns_pool.tile([128, D], F32, tag=f"x{qs}", name="x")
                        nc.vector.tensor_add(x[:], pf_l[qs][:], qn_t[qs][:])
                        x_l.append(x)

                    # q-subtile 0's chain leads so the LN ACT stream starts
                    # while the other three fc groups still run on the PE
                    fc_qs(0)
                    x_qs(0)
                    for qs in range(1, 4):
                        fc_qs(qs)
                    for qs in range(1, 4):
                        x_qs(qs)
                    for qs in range(4):
                        msum = lnsc_pool.tile([128, 1], F32, tag=f"msum{qs}")
                        nc.vector.reduce_sum(
                            out=msum[:], in_=x_l[qs][:], axis=mybir.AxisListType.X
                        )
                        nmu = lnsc_pool.tile([128, 1], F32, tag=f"nmu{qs}")
                        nc.scalar.activation(
                            nmu[:], msum[:], mybir.ActivationFunctionType.Copy,
                            scale=-1.0 / D,
                        )
                        nmu_l.append(nmu)
                    for qs in range(4):
                        sq = lns_pool.tile([128, D], F32, tag="t", name="sq")
                        vsum = lnsc_pool.tile([128, 1], F32, tag=f"vsum{qs}")
                        nc.scalar.activation(
                            sq[:], x_l[qs][:],
                            mybir.ActivationFunctionType.Square,
                            bias=nmu_l[qs][:], accum_out=vsum[:],
                        )
                        std = lnsc_pool.tile([128, 1], F32, tag=f"std{qs}")
                        nc.scalar.activation(
                            std[:], vsum[:],
                            mybir.ActivationFunctionType.Sqrt,
                            scale=1.0 / D, bias=epst[:],
                        )
                        rstd = lnsc_pool.tile([128, 1], F32, tag=f"rstd{qs}")
                        nc.vector.reciprocal(rstd[:], std[:])
                        rstd_l.append(rstd)
                    for qs in range(4):
                        xn = lns_pool.tile([128, D], F32, tag="t", name="xn")
                        nc.vector.tensor_scalar(
                            out=xn[:], in0=x_l[qs][:], scalar1=nmu_l[qs][:],
                            scalar2=rstd_l[qs][:],
                            op0=mybir.AluOpType.add, op1=mybir.AluOpType.mult,
                        )
                        xg = lns_pool.tile([128, D], F32, tag="g", name="xg")
                        nc.gpsimd.tensor_mul(xg[:], xn[:], gbt[:])
                        xb = lns_pool.tile([128, D], F32, tag="b", name="xb")
                        nc.vector.tensor_add(xb[:], xg[:], bbt[:])
                        nc.sync.dma_start(
                            out[128 * qs : 128 * (qs + 1), :], xb[:]
                        )

_NC_CACHE = None


def kernel(q, k, v, Wq, Wk, Wv, Wfc, bfc, gamma, beta):
    global _NC_CACHE
    if _NC_CACHE is None:
        _NC_CACHE = build_kernel()
    nc = _NC_CACHE

    bf16 = ml_dtypes.bfloat16
    q = np.asarray(q, dtype=np.float32)
    k = np.asarray(k, dtype=np.float32)
    v = np.asarray(v, dtype=np.float32)
    Wq = np.ascontiguousarray(np.asarray(Wq, dtype=np.float32).astype(bf16))
    Wk = np.ascontiguousarray(np.asarray(Wk, dtype=np.float32).astype(bf16))
    Wv = np.ascontiguousarray(np.asarray(Wv, dtype=np.float32).astype(bf16))
    Wfc = np.ascontiguousarray(np.asarray(Wfc, dtype=np.float32).astype(bf16))
    bfc = np.asarray(bfc, dtype=np.float32)
    gamma = np.asarray(gamma, dtype=np.float32)
    beta = np.asarray(beta, dtype=np.float32)

    gb = np.ascontiguousarray(np.broadcast_to(gamma, (128, D)))
    bb = np.ascontiguousarray(np.broadcast_to(beta, (128, D)))

    in_maps = []
    for c in range(N_CORES):
        b, r0 = c // 4, (c % 4) * SS
        qs = q[b, r0 : r0 + SS]
        ks = k[b, r0 : r0 + SS]
        vs = v[b, r0 : r0 + SS]
        mbm = np.zeros((128, 4), np.float32)
        mbm[:, c % 4] = -30000.0
        in_maps.append(
            {
                "rk": np.array([[c % 4, 0, 0, 0]], dtype=np.int32),
                "qT": np.ascontiguousarray(qs.T.astype(bf16)),
                "kT": np.ascontiguousarray(ks.T.astype(bf16)),
                "vT": np.ascontiguousarray(vs.T.astype(bf16)),
                "qn": np.ascontiguousarray(qs + bfc),
                "Wq": Wq, "Wk": Wk, "Wv": Wv, "Wfc": Wfc,
                "gb": gb, "bb": bb, "mb": mbm,
            }
        )

    global _last_in_maps
    _last_in_maps = in_maps
    res = run_bass_kernel_spmd(nc, in_maps, list(range(N_CORES)))
    out = np.empty((B, S, D), dtype=np.float32)
    for c in range(N_CORES):
        b, r0 = c // 4, (c % 4) * SS
        out[b, r0 : r0 + SS] = res.results[c]["out"]
    return out
